# revision 12
# baseline (speedup 1.0000x reference)
"""Conv2d-via-Linear Trainium2 kernel (v2: phase-packed quad im2col).

The problem's [16,30,30,3,64,64] weight is (for the reference's
setup_inputs) a structured-sparse replication of a single 5x5/stride-2
conv kernel w0 [16,3,5,5]:  big[:, oh, ow, :, 2oh:2oh+5, 2ow:2ow+5] = w0.
So out = x2 @ w2.T + bias is exactly Conv2d(x, w0, stride=2) + b0.

Device strategy (8 NeuronCores, batch-parallel, 8 images per core):
  - Output is computed in 2x2 blocks (oh = 2oh'+i, ow = 2ow'+j), so one
    rhs column is the union receptive field of a quad. Vertical kernel
    offsets kha = kh+2i in 0..6 decompose as kha = r + 4*hc with
    r in 0..3: the hc=1 taps read the SAME physical rows at a column
    offset of one oh''-block. Horizontal offsets kwa in 0..6 are
    materialized. Physical SBUF operand: 84 rows (c, r in 0..3,
    kwa in 0..6) x 1920 cols (b, oh'' in 0..15, ow' in 0..14), i.e. a
    pure 4-phase permutation of x in H (zero vertical duplication) and
    1.75x duplication in W: 326KB fp16 per core (vs 2.4MB baseline).
  - Row 84 is constant 1.0; the hc=0 weight matrix carries the bias
    there, so bias-add happens inside the matmul.
  - Per image: 2 accumulating matmuls (hc=0: W0, rhs rows 0..14;
    hc=1: W1, rhs rows 1..15) into one PSUM region [64, 225] whose
    partitions are (i,j,o). Odd/even images go to PE column halves
    (tile_position (0,0)/(0,64)) and run concurrently.
  - Evacuation: plain PSUM->SBUF copy with fp32->fp16 cast, alternating
    DVE / ACT engines; two [128,450] fp16 output DMAs (230KB total).

If the weight/bias do not have the replicated-conv structure (never the
case for the real reference inputs), falls back to the dense matmul on
host so the result is still correct.
"""

import numpy as np

B, C, H, W = 64, 3, 64, 64
O, KK, S = 16, 5, 2
OH = OW = 30
NCORES = 8
BPC = B // NCORES  # images per core

NROW = 84          # (c, r in 0..3, kwa in 0..6)
NP = NROW + 1      # + ones row for bias
OHQ = OWQ = 15     # quad grid
NHH = 16           # oh'' in 0..15 (one extra block for the hc=1 shift)
NCOLB = NHH * OWQ  # 240 cols per image
NCOL = BPC * NCOLB  # 1920

_NC_CACHE = {}
LAST_RESULT = None


def _install_trace_shim():
    """Make bass_utils' trace path importable even when antenv.axon_hooks
    is absent (it is in this container). Harmless if tracing is off."""
    import sys, types
    try:
        import antenv.axon_hooks  # noqa: F401
        return
    except ImportError:
        pass
    mod = types.ModuleType("antenv.axon_hooks")
    hook = [None]
    mod.set_axon_ntff_profile_hook = lambda h: hook.__setitem__(0, h)
    mod.get_axon_ntff_profile_hook = lambda: hook[0]
    sys.modules["antenv.axon_hooks"] = mod
    try:
        from trn_agent_boot.trn_boot import _ntff_profile_via_ctypes
        hook[0] = _ntff_profile_via_ctypes("/opt/axon/libaxon_pjrt.so")
    except Exception:
        pass


def _structure_ok(weight, w0, bias, b0):
    """Exact check that `weight` is w0 replicated per output position and
    everything else zero, and that bias is b0 repeated per position."""
    try:
        from numpy.lib.stride_tricks import as_strided
        s = weight.strides
        blocks = as_strided(
            weight,
            shape=(OH, OW, O, C, KK, KK),
            strides=(s[1] + S * s[4], s[2] + S * s[5], s[0], s[3], s[4], s[5]),
        )
        if not (blocks == w0[None, None]).all():
            return False
        if np.count_nonzero(weight) != OH * OW * np.count_nonzero(w0):
            return False
        if not (bias[0].reshape(O, OH * OW) == b0[:, None]).all():
            return False
        return True
    except Exception:
        return False


def _build_nc():
    import concourse.mybir as mybir
    import concourse.tile as tile
    from concourse import bacc

    f32 = mybir.dt.float32
    f16 = mybir.dt.float16
    nc = bacc.Bacc(None, target_bir_lowering=False)
    with tile.TileContext(nc) as tc:
        with tc.tile_pool(name="dram", bufs=1, space="DRAM") as dram:
            # 128-partition operand: DMA engine fan-out is partition-based
            # (85-row transfers only engage 5 of 16 SDMA engines; 128-row
            # transfers engage all 16), so rows 85..127 are zero padding.
            # Layout: cols 0..127 = weights (W0|W1), cols 128.. = x data;
            # folding weights in saves one ~0.65us dma_start issue slot.
            xin = dram.tile([128, 128 + NCOL], f16, kind="ExternalInput",
                            name="xin", uniquify=False)
            out = dram.tile([128, 900], f16, kind="ExternalOutput",
                            name="out", uniquify=False)

            with (
                tc.tile_pool(name="xdata", bufs=1) as xpool,
                tc.tile_pool(name="evac", bufs=1) as evacp,
                tc.tile_pool(name="psum", bufs=4, space="PSUM") as psump,
                tc.tile_pool(name="wpsum", bufs=1, space="PSUM") as wpsump,
            ):
                x_sb = xpool.tile([128, 128 + NCOL], f16, name="xsb")
                # dummy-matmul operand (zeroed scratch, no DMA dependency)
                warm = xpool.tile([128, 512], f16, name="warm")
                nc.gpsimd.memset(warm[:], 0.0)

                # two input chunks on the SP ring: weights + images 0..3,
                # then images 4..7
                nc.sync.dma_start(x_sb[:, 0:1088], xin[:, 0:1088])
                nc.sync.dma_start(x_sb[:, 1088:2048], xin[:, 1088:2048])

                # PE warm-up: the HAM clock gate holds the PE at 1.2 GHz
                # until it has been busy ~3.4us. The first real matmul can't
                # start until chunk0's DMA-completion sem (~2.7us into the
                # body), so burn that dead window with dummy matmuls to
                # reach 2.4 GHz before the real ones issue.
                wps = wpsump.tile([64, 512], f32, name="wps")
                for d in range(6):
                    nc.tensor.matmul(
                        wps[:, :], warm[0:NP, 0:64], warm[0:NP, :],
                        start=True, stop=True, skip_group_check=True,
                        tile_position=(0, 0),
                    )
                for d in range(2):
                    nc.tensor.matmul(
                        wps[:, 0:128], warm[0:NP, 0:64], warm[0:NP, 0:128],
                        start=True, stop=True, skip_group_check=True,
                        tile_position=(0, 0),
                    )

                xv = x_sb[:, 128:].rearrange("p (b h w) -> p b h w",
                                             b=BPC, h=NHH, w=OWQ)

                ev = evacp.tile([128, 900], f16, name="ev")
                for p in range(4):  # image pairs
                    ps = psump.tile([128, 225], f32, tag="ps")
                    for h in range(2):
                        m = 2 * p + h
                        nc.tensor.matmul(
                            ps[64 * h:64 * h + 64, :],
                            x_sb[0:NP, 0:64],
                            xv[0:NP, m, 0:OHQ, :],
                            start=True, stop=False,
                            skip_group_check=True,
                            tile_position=(0, 64 * h),
                        )
                        nc.tensor.matmul(
                            ps[64 * h:64 * h + 64, :],
                            x_sb[0:NP, 64:128],
                            xv[0:NP, m, 1:1 + OHQ, :],
                            start=False, stop=True,
                            skip_group_check=True,
                            tile_position=(0, 64 * h),
                        )
                    dst = ev[:, 225 * p:225 * p + 225]
                    if p % 2 == 0:
                        nc.vector.tensor_copy(dst, ps[:, :])
                    else:
                        nc.scalar.activation(
                            dst, ps[:, :], mybir.ActivationFunctionType.Copy)
                        nc.sync.dma_start(
                            out[:, 450 * (p // 2):450 * (p // 2) + 450],
                            ev[:, 450 * (p // 2):450 * (p // 2) + 450],
                        )
    nc.compile()
    return nc


def _host_fallback(x, weight, bias):
    x2 = x.reshape(B, -1)
    w2 = np.asarray(weight, dtype=np.float32).reshape(O * OH * OW, -1)
    return (x2 @ w2.T + bias).reshape(B, O, OH, OW).astype(np.float32)


def kernel(x, weight, bias):
    global LAST_RESULT
    x = np.ascontiguousarray(np.asarray(x), dtype=np.float32)
    weight = np.asarray(weight)
    bias = np.ascontiguousarray(np.asarray(bias), dtype=np.float32)

    w0 = np.ascontiguousarray(weight[:, 0, 0, :, :KK, :KK], dtype=np.float32)
    b0 = bias[0].reshape(O, OH * OW)[:, 0].copy()

    if not _structure_ok(weight, w0, bias, b0):
        return _host_fallback(x, weight, bias)

    _install_trace_shim()

    # host layout prep --------------------------------------------------
    # phys[core][c, r, kwa][b, hh, ow'] = x[8*core+b, c, 4*hh+r, 4*ow'+kwa]
    xs = x.reshape(NCORES, BPC, C, NHH, 4, 64)
    wcols = 4 * np.arange(OWQ)[None, :] + np.arange(7)[:, None]  # [kwa, ow']
    xg = xs[..., wcols]                      # [core, b, c, hh, r, kwa, ow']
    phys = xg.transpose(0, 2, 4, 5, 1, 3, 6)  # core, c, r, kwa, b, hh, ow'
    xin = np.zeros((NCORES, 128, 128 + NCOL), dtype=np.float16)
    xin[:, :NROW, 128:] = phys.reshape(NCORES, NROW, NCOL)
    xin[:, NROW, 128:] = 1.0

    # weights: W[hc][p=(c,r,kwa), 16*(2i+j)+o] = w0[o, c, r+4hc-2i, kwa-2j]
    Wf = np.zeros((2, 128, 64), dtype=np.float32)
    for hc in range(2):
        for i in range(2):
            for j in range(2):
                q = 2 * i + j
                for r in range(4):
                    kh = r + 4 * hc - 2 * i
                    if not (0 <= kh < KK):
                        continue
                    for kwa in range(7):
                        kw = kwa - 2 * j
                        if not (0 <= kw < KK):
                            continue
                        for c in range(C):
                            pp = c * 28 + r * 7 + kwa
                            Wf[hc, pp, 16 * q:16 * q + 16] = w0[:, c, kh, kw]
    for q in range(4):
        Wf[0, NROW, 16 * q:16 * q + 16] = b0
    xin[:, :, 0:64] = Wf[0].astype(np.float16)[None]
    xin[:, :, 64:128] = Wf[1].astype(np.float16)[None]

    # device run --------------------------------------------------------
    if "nc" not in _NC_CACHE:
        _NC_CACHE["nc"] = _build_nc()
    nc = _NC_CACHE["nc"]

    from concourse.bass_utils import run_bass_kernel_spmd

    in_maps = [{"xin": xin[i]} for i in range(NCORES)]
    res = run_bass_kernel_spmd(nc, in_maps, core_ids=list(range(NCORES)))
    LAST_RESULT = res

    # unpack: dev[64h+16(2i+j)+o, 225*p + 15*oh' + ow']
    #   -> y[8*core + 2p+h, o, 2oh'+i, 2ow'+j]
    y = np.empty((B, O, OH, OW), dtype=np.float32)
    for core in range(NCORES):
        dv = np.asarray(res.results[core]["out"], dtype=np.float32)
        dv = dv.reshape(2, 2, 2, O, 4, OHQ, OWQ)  # h,i,j,o,p,oh',ow'
        for h in range(2):
            for p in range(4):
                m = 2 * p + h
                for i in range(2):
                    for j in range(2):
                        y[8 * core + m, :, i::2, j::2] = dv[h, i, j, :, p]
    return y
